# revision 29
# baseline (speedup 1.0000x reference)
"""Trainium2 Bass kernel: conv3d(3->16, 3x3x3, SAME) + bias + hardswish +
spatial mean + per-channel affine, for x of shape (8, 3, 32, 128, 128).

Sharding: data-parallel over batch B=8 across 8 NeuronCores (one batch
element per core, no collectives).

Per-core formulation:
  - H is split into 16 blocks of 8 output rows; each block is one SBUF slab
    [90 partitions = (kd, ci, hh in 0..9), d_pad=34, w_pad=130] in fp16,
    prebuilt HOST-side (halos zeroed, kd-replicas stored shifted by 2-kd
    planes along padded d) so each block loads with a single large DMA and
    all conv boundary handling is data-side.
  - lhsT[(kd,ci,hh)=90, (hhat,co)=128] per kw; the kh taps live in the
    stationary's (hh, hhat) band; the 3 kw taps are 3 PSUM-accumulating
    matmuls whose rhs is the slab AP shifted by kw along w (zero-copy).
  - PSUM tile [128=(hhat,co), 16 d-planes, 128 w] spans 4 banks (each
    N=512 matmul fills one bank); drained in 2048-column ops:
    ACT: c = Identity(psum + bias) -> fp16
    DVE: t = clip(c, -3, 3); p = c*t; tensor_scalar identity pass with
    accum_out delivering sum(c*t)  [hardswish(v)*6 = v*clip(v,-3,3) + 3v]
  - sum(v) is linear in x, computed exactly host-side from 27 box sums;
    host combines m = (sum(c*t) + 3*sum(v)) / 6 / (D*H*W), then the
    running-stats affine.
"""

import numpy as np

import concourse.bass as bass
import concourse.tile as tile
from concourse import mybir
from concourse.bass_utils import run_bass_kernel_spmd

B, CI, CO, D, H, W = 8, 3, 16, 32, 128, 128
DP, WP = D + 2, W + 2
EPS = 1e-5
NHB = 16          # h-blocks of 8 output rows
NHALF = 2         # psum tiles per h-block (16 d-planes each)
NTILES = NHB * NHALF  # 32 drain tiles

_f32 = mybir.dt.float32
_bf16 = mybir.dt.bfloat16
_f16 = mybir.dt.float16


_SPLIT_ENGINES = None


def _split_sync_waits(nc, max_waits=1):
    """This walrus build rejects instructions carrying more than one sync
    wait ("Too many sync wait commands"). Tile attaches one wait per sem
    domain to the consuming instruction; hoist the excess onto preceding
    same-engine NOPs (wait semantics are AND across program order)."""
    global _SPLIT_ENGINES
    if _SPLIT_ENGINES is None:
        _SPLIT_ENGINES = {
            mybir.EngineType.PE,
            mybir.EngineType.Activation,
            mybir.EngineType.DVE,
            mybir.EngineType.SP,
            mybir.EngineType.Pool,
        }
    ctr = 0
    for f in nc.m.functions:
        for blk in f.blocks:
            insts = blk.instructions
            if not any(
                i.sync_info is not None
                and i.sync_info.on_wait
                and len(i.sync_info.on_wait) > max_waits
                and i.engine in _SPLIT_ENGINES
                for i in insts
            ):
                continue
            out = []
            for inst in insts:
                si = inst.sync_info
                if (
                    si is not None
                    and si.on_wait
                    and len(si.on_wait) > max_waits
                    and inst.engine in _SPLIT_ENGINES
                ):
                    waits = list(si.on_wait)
                    si.on_wait.clear()
                    si.on_wait.extend(waits[:max_waits])
                    for w in waits[max_waits:]:
                        n = mybir.InstNoOp(name=f"waitsplit_{ctr}", ins=[], outs=[])
                        ctr += 1
                        n.engine = inst.engine
                        n.bass_nofuse = True
                        n.sync_info = mybir.SyncInfo(on_wait=[w], on_update=[])
                        out.append(n)
                out.append(inst)
            blk.instructions[:] = out


def _build_nc(reps=1):
    nc = bass.Bass(
        "TRN2",
        target_bir_lowering=False,
        debug=False,
        enable_asserts=False,
        num_devices=B,
    )
    xs = nc.dram_tensor("xs", [NHB, 90, DP, WP], _f16, kind="ExternalInput")
    wt = nc.dram_tensor("wt", [128, 3, 128], _f16, kind="ExternalInput")
    bias_d = nc.dram_tensor("bias", [128, 1], _f32, kind="ExternalInput")
    out_d = nc.dram_tensor("out", [128, 1], _f32, kind="ExternalOutput")

    Ident = mybir.ActivationFunctionType.Identity
    Alu = mybir.AluOpType

    with tile.TileContext(nc) as tc:
        with (
            tc.tile_pool(name="singles", bufs=1) as singles,
            tc.tile_pool(name="slabs", bufs=3) as slabs,
            tc.tile_pool(name="psum", bufs=2, space="PSUM") as psump,
            tc.tile_pool(name="dr", bufs=4) as drpool,
        ):
            wt_sb = singles.tile([128, 3, 128], _f16)
            nc.sync.dma_start(out=wt_sb, in_=wt[:, :, :])
            bias_sb = singles.tile([128, 1], _f32)
            nc.sync.dma_start(out=bias_sb, in_=bias_d[:, :])
            Szt = singles.tile([128, NTILES], _f32)
            nc.gpsimd.memset(Szt, 0.0)

            csb = None
            for hb in range(NHB * reps):
                hb = hb % NHB
                slab = slabs.tile([90, DP, WP], _f16, tag="slab")
                nc.sync.dma_start(out=slab[:, :, :], in_=xs[hb, 0:90, :, :])

                for half in range(NHALF):
                    pst = psump.tile([128, 16, 128], _f32, tag="ps")
                    for c4 in range(4):
                        dstart = 16 * half + 4 * c4
                        for kw in range(3):
                            nc.tensor.matmul(
                                pst[:, 4 * c4 : 4 * c4 + 4, :],
                                wt_sb[0:90, kw, :],
                                slab[0:90, dstart + 1 : dstart + 5, kw : kw + 128],
                                start=(kw == 0),
                                stop=(kw == 2),
                            )
                    idx = hb * NHALF + half
                    csb = drpool.tile([128, 16, 128], _f16, tag="c")
                    nc.scalar.activation(
                        out=csb,
                        in_=pst[:, :, :],
                        func=Ident,
                        bias=bias_sb[:, 0:1],
                        scale=1.0,
                    )
                    tsb = drpool.tile([128, 16, 128], _f16, tag="t")
                    nc.vector.tensor_scalar(
                        out=tsb, in0=csb, scalar1=-3.0, scalar2=3.0,
                        op0=Alu.max, op1=Alu.min,
                    )
                    psb = drpool.tile([128, 16, 128], _f16, tag="p")
                    nc.vector.tensor_tensor(out=psb, in0=csb, in1=tsb, op=Alu.mult)
                    ssb = drpool.tile([128, 16, 128], _f16, tag="s")
                    # single-scalar-op tensor_scalar runs in 4x mode; its
                    # accum_out delivers sum(c*t)
                    nc.vector.tensor_scalar(
                        out=ssb,
                        in0=psb,
                        scalar1=1.0,
                        scalar2=0.0,
                        op0=Alu.mult,
                        op1=Alu.add,
                        accum_out=Szt[:, idx : idx + 1],
                    )

            out_sb = singles.tile([128, 1], _f32)
            nc.vector.tensor_reduce(
                out=out_sb[:, 0:1], in_=Szt, axis=mybir.AxisListType.X, op=Alu.add
            )
            nc.sync.dma_start(out=out_d[:, :], in_=out_sb)
    _split_sync_waits(nc)
    return nc


_NC = None


def _get_nc():
    global _NC
    if _NC is None:
        _NC = _build_nc()
    return _NC


def _prep_weights(weight):
    """lhsT[(kd,ci,hh)=90 (pad 128), kw=3, (hhat,co)=128] in fp16."""
    wt = np.zeros((128, 3, 128), np.float32)
    w = np.asarray(weight, np.float32)  # [co, ci, kd, kh, kw]
    for kd in range(3):
        for ci in range(CI):
            for hhat in range(8):
                for kh in range(3):
                    hh = hhat + kh
                    wt[30 * kd + 10 * ci + hh, :, 16 * hhat : 16 * hhat + CO] = (
                        w[:, ci, kd, kh, :].transpose(1, 0)
                    )
    return wt.astype(np.float16)


def _prep_slabs(xb):
    """Host im2col-lite: [NHB, 90, DP, WP] fp16 with zero halos and the kd
    replicas stored shifted by (2-kd) planes along padded d."""
    xb16 = np.asarray(xb, np.float32).astype(np.float16)
    slabs = np.zeros((NHB, 90, DP, WP), np.float16)
    for kd in range(3):
        for ci in range(CI):
            p0 = 30 * kd + 10 * ci
            for hh in range(10):
                # h = 8*hb - 1 + hh for hb in 0..16 -> vectorize over hb
                hs = np.arange(NHB) * 8 - 1 + hh
                valid = (hs >= 0) & (hs < H)
                # advanced index on h fronts that axis: (n_valid, D, W)
                slabs[valid, p0 + hh, 2 - kd : DP - kd, 1 : W + 1] = xb16[
                    ci, :, hs[valid], :
                ]
    return slabs


def _host_sum_v(x, weight, bias):
    """Exact sum over all pixels of (conv3d(x) + bias) per (b, co): linear in
    x, so computed host-side from 27 boundary-truncated box sums."""
    x64 = np.asarray(x, np.float64)
    w64 = np.asarray(weight, np.float64)
    sl = {0: slice(0, -1), 1: slice(None), 2: slice(1, None)}
    box = np.empty((3, 3, 3, B, CI), np.float64)  # [kd, kh, kw, b, ci]
    for kw in range(3):
        xw = x64[:, :, :, :, sl[kw]].sum(axis=4)  # [B, CI, D, H]
        for kh in range(3):
            xh = xw[:, :, :, sl[kh]].sum(axis=3)  # [B, CI, D]
            for kd in range(3):
                box[kd, kh, kw] = xh[:, :, sl[kd]].sum(axis=2)
    sv = np.einsum("oidhw,dhwbi->bo", w64, box)
    sv += np.asarray(bias, np.float64)[None, :] * (D * H * W)
    return sv


def kernel(x, weight, bias, running_mean, running_var, gamma, beta):
    nc = _get_nc()
    wt16 = _prep_weights(weight)
    bias_arr = np.tile(np.asarray(bias, np.float32), 8).reshape(128, 1)
    x_np = np.asarray(x, np.float32)
    in_maps = [
        {"xs": _prep_slabs(x_np[b]), "wt": wt16, "bias": bias_arr}
        for b in range(B)
    ]
    res = run_bass_kernel_spmd(nc, in_maps, core_ids=list(range(B)))
    outs = np.stack([r["out"] for r in res.results])  # [B, 128, 1]
    sv = _host_sum_v(x, weight, bias)  # [B, CO]
    s6 = outs[:, :, 0].reshape(B, 8, CO).sum(axis=1) + 3.0 * sv
    m = s6 / (6.0 * D * H * W)
    rm = np.asarray(running_mean, np.float32)
    rv = np.asarray(running_var, np.float32)
    g = np.asarray(gamma, np.float32)
    bt = np.asarray(beta, np.float32)
    out = (m - rm[None, :]) / np.sqrt(rv + EPS)[None, :] * g[None, :] + bt[None, :]
    return out.astype(np.float32)


# revision 30
# speedup vs baseline: 1.0088x; 1.0088x over previous
"""Trainium2 Bass kernel: conv3d(3->16, 3x3x3, SAME) + bias + hardswish +
spatial mean + per-channel affine, for x of shape (8, 3, 32, 128, 128).

Sharding: data-parallel over batch B=8 across 8 NeuronCores (one batch
element per core, no collectives).

Per-core formulation:
  - H is split into 16 blocks of 8 output rows; each block is one SBUF slab
    [90 partitions = (kd, ci, hh in 0..9), d_pad=34, w_pad=130] in fp16,
    prebuilt HOST-side (halos zeroed, kd-replicas stored shifted by 2-kd
    planes along padded d) so each block loads with a single large DMA and
    all conv boundary handling is data-side.
  - lhsT[(kd,ci,hh)=90, (hhat,co)=128] per kw; the kh taps live in the
    stationary's (hh, hhat) band; the 3 kw taps are 3 PSUM-accumulating
    matmuls whose rhs is the slab AP shifted by kw along w (zero-copy).
  - PSUM tile [128=(hhat,co), 16 d-planes, 128 w] spans 4 banks (each
    N=512 matmul fills one bank); drained in 2048-column ops:
    ACT: c = Identity(psum + bias) -> fp16
    DVE: t = clip(c, -3, 3); p = c*t; tensor_scalar identity pass with
    accum_out delivering sum(c*t)  [hardswish(v)*6 = v*clip(v,-3,3) + 3v]
  - sum(v) is linear in x, computed exactly host-side from 27 box sums;
    host combines m = (sum(c*t) + 3*sum(v)) / 6 / (D*H*W), then the
    running-stats affine.
"""

import numpy as np

import concourse.bass as bass
import concourse.tile as tile
from concourse import mybir
from concourse.bass_utils import run_bass_kernel_spmd

B, CI, CO, D, H, W = 8, 3, 16, 32, 128, 128
DP, WP = D + 2, W + 2
EPS = 1e-5
NHB = 16          # h-blocks of 8 output rows
NHALF = 2         # psum tiles per h-block (16 d-planes each)
NTILES = NHB * NHALF  # 32 drain tiles

_f32 = mybir.dt.float32
_bf16 = mybir.dt.bfloat16
_f16 = mybir.dt.float16


_SPLIT_ENGINES = None


def _split_sync_waits(nc, max_waits=1):
    """This walrus build rejects instructions carrying more than one sync
    wait ("Too many sync wait commands"). Tile attaches one wait per sem
    domain to the consuming instruction; hoist the excess onto preceding
    same-engine NOPs (wait semantics are AND across program order)."""
    global _SPLIT_ENGINES
    if _SPLIT_ENGINES is None:
        _SPLIT_ENGINES = {
            mybir.EngineType.PE,
            mybir.EngineType.Activation,
            mybir.EngineType.DVE,
            mybir.EngineType.SP,
            mybir.EngineType.Pool,
        }
    ctr = 0
    for f in nc.m.functions:
        for blk in f.blocks:
            insts = blk.instructions
            if not any(
                i.sync_info is not None
                and i.sync_info.on_wait
                and len(i.sync_info.on_wait) > max_waits
                and i.engine in _SPLIT_ENGINES
                for i in insts
            ):
                continue
            out = []
            for inst in insts:
                si = inst.sync_info
                if (
                    si is not None
                    and si.on_wait
                    and len(si.on_wait) > max_waits
                    and inst.engine in _SPLIT_ENGINES
                ):
                    waits = list(si.on_wait)
                    si.on_wait.clear()
                    si.on_wait.extend(waits[:max_waits])
                    for w in waits[max_waits:]:
                        n = mybir.InstNoOp(name=f"waitsplit_{ctr}", ins=[], outs=[])
                        ctr += 1
                        n.engine = inst.engine
                        n.bass_nofuse = True
                        n.sync_info = mybir.SyncInfo(on_wait=[w], on_update=[])
                        out.append(n)
                out.append(inst)
            blk.instructions[:] = out


def _build_nc(reps=1):
    nc = bass.Bass(
        "TRN2",
        target_bir_lowering=False,
        debug=False,
        enable_asserts=False,
        num_devices=B,
    )
    xs = nc.dram_tensor("xs", [NHB, 90, DP, WP], _f16, kind="ExternalInput")
    wt = nc.dram_tensor("wt", [128, 3, 128], _f16, kind="ExternalInput")
    bias_d = nc.dram_tensor("bias", [128, 1], _f32, kind="ExternalInput")
    out_d = nc.dram_tensor("out", [128, 1], _f32, kind="ExternalOutput")

    Ident = mybir.ActivationFunctionType.Identity
    Alu = mybir.AluOpType

    with tile.TileContext(nc) as tc:
        with (
            tc.tile_pool(name="singles", bufs=1) as singles,
            tc.tile_pool(name="slabs", bufs=3) as slabs,
            tc.tile_pool(name="psum", bufs=2, space="PSUM") as psump,
            tc.tile_pool(name="dr", bufs=4) as drpool,
        ):
            wt_sb = singles.tile([128, 3, 128], _f16)
            nc.sync.dma_start(out=wt_sb, in_=wt[:, :, :])
            bias_sb = singles.tile([128, 1], _f32)
            nc.sync.dma_start(out=bias_sb, in_=bias_d[:, :])
            Szt = singles.tile([128, NTILES], _f32)
            nc.gpsimd.memset(Szt, 0.0)

            csb = None
            for hb in range(NHB * reps):
                hb = hb % NHB
                slab = slabs.tile([90, DP, WP], _f16, tag="slab")
                # split the load so the first half's matmuls (dpad < 21)
                # can start before the d-tail arrives
                nc.sync.dma_start(out=slab[:, 0:21, :], in_=xs[hb, 0:90, 0:21, :])
                nc.sync.dma_start(out=slab[:, 21:DP, :], in_=xs[hb, 0:90, 21:DP, :])

                for half in range(NHALF):
                    pst = psump.tile([128, 16, 128], _f32, tag="ps")
                    for c4 in range(4):
                        dstart = 16 * half + 4 * c4
                        for kw in range(3):
                            nc.tensor.matmul(
                                pst[:, 4 * c4 : 4 * c4 + 4, :],
                                wt_sb[0:90, kw, :],
                                slab[0:90, dstart + 1 : dstart + 5, kw : kw + 128],
                                start=(kw == 0),
                                stop=(kw == 2),
                            )
                    idx = hb * NHALF + half
                    csb = drpool.tile([128, 16, 128], _f16, tag="c")
                    nc.scalar.activation(
                        out=csb,
                        in_=pst[:, :, :],
                        func=Ident,
                        bias=bias_sb[:, 0:1],
                        scale=1.0,
                    )
                    tsb = drpool.tile([128, 16, 128], _f16, tag="t")
                    nc.vector.tensor_scalar(
                        out=tsb, in0=csb, scalar1=-3.0, scalar2=3.0,
                        op0=Alu.max, op1=Alu.min,
                    )
                    psb = drpool.tile([128, 16, 128], _f16, tag="p")
                    nc.vector.tensor_tensor(out=psb, in0=csb, in1=tsb, op=Alu.mult)
                    ssb = drpool.tile([128, 16, 128], _f16, tag="s")
                    # single-scalar-op tensor_scalar runs in 4x mode; its
                    # accum_out delivers sum(c*t)
                    nc.vector.tensor_scalar(
                        out=ssb,
                        in0=psb,
                        scalar1=1.0,
                        scalar2=0.0,
                        op0=Alu.mult,
                        op1=Alu.add,
                        accum_out=Szt[:, idx : idx + 1],
                    )

            out_sb = singles.tile([128, 1], _f32)
            nc.vector.tensor_reduce(
                out=out_sb[:, 0:1], in_=Szt, axis=mybir.AxisListType.X, op=Alu.add
            )
            nc.sync.dma_start(out=out_d[:, :], in_=out_sb)
    _split_sync_waits(nc)
    return nc


_NC = None


def _get_nc():
    global _NC
    if _NC is None:
        _NC = _build_nc()
    return _NC


def _prep_weights(weight):
    """lhsT[(kd,ci,hh)=90 (pad 128), kw=3, (hhat,co)=128] in fp16."""
    wt = np.zeros((128, 3, 128), np.float32)
    w = np.asarray(weight, np.float32)  # [co, ci, kd, kh, kw]
    for kd in range(3):
        for ci in range(CI):
            for hhat in range(8):
                for kh in range(3):
                    hh = hhat + kh
                    wt[30 * kd + 10 * ci + hh, :, 16 * hhat : 16 * hhat + CO] = (
                        w[:, ci, kd, kh, :].transpose(1, 0)
                    )
    return wt.astype(np.float16)


def _prep_slabs(xb):
    """Host im2col-lite: [NHB, 90, DP, WP] fp16 with zero halos and the kd
    replicas stored shifted by (2-kd) planes along padded d."""
    xb16 = np.asarray(xb, np.float32).astype(np.float16)
    slabs = np.zeros((NHB, 90, DP, WP), np.float16)
    for kd in range(3):
        for ci in range(CI):
            p0 = 30 * kd + 10 * ci
            for hh in range(10):
                # h = 8*hb - 1 + hh for hb in 0..16 -> vectorize over hb
                hs = np.arange(NHB) * 8 - 1 + hh
                valid = (hs >= 0) & (hs < H)
                # advanced index on h fronts that axis: (n_valid, D, W)
                slabs[valid, p0 + hh, 2 - kd : DP - kd, 1 : W + 1] = xb16[
                    ci, :, hs[valid], :
                ]
    return slabs


def _host_sum_v(x, weight, bias):
    """Exact sum over all pixels of (conv3d(x) + bias) per (b, co): linear in
    x, so computed host-side from 27 boundary-truncated box sums."""
    x64 = np.asarray(x, np.float64)
    w64 = np.asarray(weight, np.float64)
    sl = {0: slice(0, -1), 1: slice(None), 2: slice(1, None)}
    box = np.empty((3, 3, 3, B, CI), np.float64)  # [kd, kh, kw, b, ci]
    for kw in range(3):
        xw = x64[:, :, :, :, sl[kw]].sum(axis=4)  # [B, CI, D, H]
        for kh in range(3):
            xh = xw[:, :, :, sl[kh]].sum(axis=3)  # [B, CI, D]
            for kd in range(3):
                box[kd, kh, kw] = xh[:, :, sl[kd]].sum(axis=2)
    sv = np.einsum("oidhw,dhwbi->bo", w64, box)
    sv += np.asarray(bias, np.float64)[None, :] * (D * H * W)
    return sv


def kernel(x, weight, bias, running_mean, running_var, gamma, beta):
    nc = _get_nc()
    wt16 = _prep_weights(weight)
    bias_arr = np.tile(np.asarray(bias, np.float32), 8).reshape(128, 1)
    x_np = np.asarray(x, np.float32)
    in_maps = [
        {"xs": _prep_slabs(x_np[b]), "wt": wt16, "bias": bias_arr}
        for b in range(B)
    ]
    res = run_bass_kernel_spmd(nc, in_maps, core_ids=list(range(B)))
    outs = np.stack([r["out"] for r in res.results])  # [B, 128, 1]
    sv = _host_sum_v(x, weight, bias)  # [B, CO]
    s6 = outs[:, :, 0].reshape(B, 8, CO).sum(axis=1) + 3.0 * sv
    m = s6 / (6.0 * D * H * W)
    rm = np.asarray(running_mean, np.float32)
    rv = np.asarray(running_var, np.float32)
    g = np.asarray(gamma, np.float32)
    bt = np.asarray(beta, np.float32)
    out = (m - rm[None, :]) / np.sqrt(rv + EPS)[None, :] * g[None, :] + bt[None, :]
    return out.astype(np.float32)
